# revision 9
# baseline (speedup 1.0000x reference)
# Trainium2 Bass kernel for nn_EquivariantBlock (LN -> rolled-concat -> MLP -> residual).
#
# Math (see reference):
#   y = LayerNorm(x) over width
#   h[b,k,:]   = sum_j y[b,(j+k)%4,:] @ W1[j*W:(j+1)*W, :] + b1      (j = (c-k)%4 for channel c)
#   out[b,k,:] = gelu_tanh(h) @ W2 + b2
#   return x + out
#
# Strategy: pure data-parallel over batch across 8 NeuronCores (weights replicated).
# Per core (1024 batch rows), process chunks of 256 rows:
#   A) LN in token-major layout (bn_stats/bn_aggr), cast bf16, DMA-transpose to
#      feature-major yT[feat128, c, ft, tok256].
#   B) For each mid tile (32): mm1 accumulates 32 K-subtiles into PSUM, where the
#      "roll" is just W1-block reindexing (j = (c-k)%4); gelu+b1 on ScalarE from
#      PSUM into a full-mid hT[mid128, k, mid_i, tok].
#   C) mm2: for each (k, out-tile): accumulate all 32 mid tiles in PSUM, then
#      fused residual+bias: out = x + psum + b2, DMA out.
# Matmuls run in bf16 (fp32 accumulation in PSUM).

import numpy as np
import ml_dtypes

BATCH = 8192
WIDTH = 1024
MID = 4096
N_CORES = 8
BPC = BATCH // N_CORES  # batch rows per core
LN_EPS = 1e-5

_CACHE = {}

# test.py hooks
TRACE = False
LAST_RESULTS = None


def _emit(nc, tc, io, bpc, b_chunk, gelu_func=None):
    """Emit the per-core program. io: dict of DRAM APs (x, ln_scale, ln_offset,
    w1, b1, w2, b2, out)."""
    from contextlib import ExitStack

    import concourse.bass as bass
    import concourse.mybir as mybir

    f32 = mybir.dt.float32
    bf16 = mybir.dt.bfloat16

    x, out = io["x"], io["out"]
    ln_scale, ln_offset = io["ln_scale"], io["ln_offset"]
    w1, b1, w2, b2 = io["w1"], io["b1"], io["w2"], io["b2"]

    if gelu_func is None:
        gelu_func = mybir.ActivationFunctionType.Gelu_apprx_tanh

    n_chunks = bpc // b_chunk
    n_bt = b_chunk // 128  # 128-row tiles per chunk per channel
    sub_op = mybir.AluOpType.subtract
    mult_op = mybir.AluOpType.mult

    def bcast_ap(src_ap, parts=128):
        # DRAM AP replicated across `parts` partitions (stride-0 partition dim).
        return bass.AP(tensor=src_ap.tensor, offset=src_ap.offset,
                       ap=[[0, parts]] + [list(d) for d in src_ap.ap])

    with ExitStack() as ctx:
        singles = ctx.enter_context(tc.tile_pool(name="singles", bufs=1))
        xpool = ctx.enter_context(tc.tile_pool(name="xpool", bufs=4))
        ypool = ctx.enter_context(tc.tile_pool(name="ypool", bufs=3))
        stats = ctx.enter_context(tc.tile_pool(name="stats", bufs=4))
        yT_pool = ctx.enter_context(tc.tile_pool(name="yT", bufs=2))
        hT_pool = ctx.enter_context(tc.tile_pool(name="hT", bufs=1))
        w1_pool = ctx.enter_context(tc.tile_pool(name="w1p", bufs=2))
        w2_pool = ctx.enter_context(tc.tile_pool(name="w2p", bufs=4))
        psum1 = ctx.enter_context(tc.tile_pool(name="psum1", bufs=2, space="PSUM"))
        psum2 = ctx.enter_context(tc.tile_pool(name="psum2", bufs=6, space="PSUM"))

        # --- constants ---
        scale_bc = singles.tile([128, WIDTH], f32)
        nc.gpsimd.dma_start(out=scale_bc[:], in_=bcast_ap(ln_scale))
        offset_bc = singles.tile([128, WIDTH], f32)
        nc.gpsimd.dma_start(out=offset_bc[:], in_=bcast_ap(ln_offset))
        b2_bc = singles.tile([128, WIDTH], f32)
        nc.gpsimd.dma_start(out=b2_bc[:], in_=bcast_ap(b2))
        b1_p = singles.tile([128, 32], f32)
        nc.sync.dma_start(out=b1_p[:], in_=b1.rearrange("(o p) -> p o", p=128))
        eps_t = singles.tile([128, 1], f32)
        nc.vector.memset(eps_t[:], LN_EPS)

        for chunk in range(n_chunks):
            b0 = chunk * b_chunk

            # ---------- Phase A: LayerNorm + transpose ----------
            yT = yT_pool.tile([128, 4, 8, b_chunk], bf16)
            for c in range(4):
                for bt in range(n_bt):
                    r0 = b0 + bt * 128
                    xt = xpool.tile([128, WIDTH], f32)
                    nc.sync.dma_start(out=xt[:], in_=x[r0:r0 + 128, c, :])
                    st = stats.tile([128, 2, 6], f32)
                    nc.vector.bn_stats(out=st[:, 0, :], in_=xt[:, 0:512])
                    nc.vector.bn_stats(out=st[:, 1, :], in_=xt[:, 512:1024])
                    mv = stats.tile([128, 2], f32)
                    nc.vector.bn_aggr(out=mv[:], in_=st[:])
                    rstd = stats.tile([128, 1], f32)
                    nc.scalar.activation(
                        out=rstd[:], in_=mv[:, 1:2],
                        func=mybir.ActivationFunctionType.Sqrt,
                        bias=eps_t[:], scale=1.0)
                    nc.vector.reciprocal(out=rstd[:], in_=rstd[:])
                    # (x - mean) * rstd, then *scale, then +offset (cast bf16)
                    nc.vector.tensor_scalar(
                        out=xt[:], in0=xt[:],
                        scalar1=mv[:, 0:1], scalar2=rstd[:],
                        op0=sub_op, op1=mult_op)
                    nc.vector.tensor_mul(out=xt[:], in0=xt[:], in1=scale_bc[:])
                    ybf = ypool.tile([128, WIDTH], bf16)
                    nc.vector.tensor_add(out=ybf[:], in0=xt[:], in1=offset_bc[:])
                    for ft in range(8):
                        nc.sync.dma_start(
                            out=yT[:, c, ft, bt * 128:(bt + 1) * 128],
                            in_=ybf[:, ft * 128:(ft + 1) * 128],
                            transpose=True)

            # ---------- Phase B: mm1 + gelu -> hT ----------
            hT = hT_pool.tile([128, 4, 32, b_chunk], bf16)
            for mid_i in range(32):
                w1t = w1_pool.tile([128, 32, 128], bf16)
                nc.sync.dma_start(out=w1t[:], in_=w1[mid_i])
                for k in range(4):
                    p1 = psum1.tile([128, b_chunk], f32)
                    n_acc = 0
                    for c in range(4):
                        j = (c - k) % 4
                        for ft in range(8):
                            nc.tensor.matmul(
                                p1[:], w1t[:, j * 8 + ft, :], yT[:, c, ft, :],
                                start=(n_acc == 0), stop=(n_acc == 31))
                            n_acc += 1
                    nc.scalar.activation(
                        out=hT[:, k, mid_i, :], in_=p1[:],
                        func=gelu_func,
                        bias=b1_p[:, mid_i:mid_i + 1], scale=1.0)

            # ---------- Phase C: mm2 + residual + store ----------
            for k in range(4):
                p2 = [[psum2.tile([128, 512], f32, name="p2", tag="p2")
                       for _ in range(2)] for _ in range(n_bt)]
                for ml in range(32):
                    w2t = w2_pool.tile([128, WIDTH], bf16)
                    nc.sync.dma_start(out=w2t[:], in_=w2[ml])
                    for sub in range(n_bt):
                        for wh in range(2):
                            nc.tensor.matmul(
                                p2[sub][wh][:],
                                hT[:, k, ml, sub * 128:(sub + 1) * 128],
                                w2t[:, wh * 512:(wh + 1) * 512],
                                start=(ml == 0), stop=(ml == 31))
                for sub in range(n_bt):
                    r0 = b0 + sub * 128
                    xr = xpool.tile([128, WIDTH], f32)
                    nc.sync.dma_start(out=xr[:], in_=x[r0:r0 + 128, k, :])
                    nc.vector.tensor_add(out=xr[:, 0:512], in0=xr[:, 0:512],
                                         in1=p2[sub][0][:])
                    nc.vector.tensor_add(out=xr[:, 512:1024], in0=xr[:, 512:1024],
                                         in1=p2[sub][1][:])
                    nc.vector.tensor_add(out=xr[:], in0=xr[:], in1=b2_bc[:])
                    nc.sync.dma_start(out=out[r0:r0 + 128, k, :], in_=xr[:])


def _build(bpc=BPC, b_chunk=256):
    """Build + compile the Bass program (one NeuronCore's SPMD program)."""
    import concourse.mybir as mybir
    import concourse.tile as tile
    from concourse import bacc

    f32 = mybir.dt.float32
    bf16 = mybir.dt.bfloat16

    nc = bacc.Bacc("TRN2", target_bir_lowering=False, debug=False,
                   enable_asserts=False)

    io = {
        "x": nc.dram_tensor("x", (bpc, 4, WIDTH), f32, kind="ExternalInput").ap(),
        "ln_scale": nc.dram_tensor("ln_scale", (WIDTH,), f32,
                                   kind="ExternalInput").ap(),
        "ln_offset": nc.dram_tensor("ln_offset", (WIDTH,), f32,
                                    kind="ExternalInput").ap(),
        # host-pretransformed: w1[mid_i, p, kt, m] = W1[kt*128+p, mid_i*128+m]
        "w1": nc.dram_tensor("w1", (32, 128, 32, 128), bf16,
                             kind="ExternalInput").ap(),
        "b1": nc.dram_tensor("b1", (MID,), f32, kind="ExternalInput").ap(),
        # w2[ml, p, w] = W2[ml*128+p, w]
        "w2": nc.dram_tensor("w2", (32, 128, WIDTH), bf16,
                             kind="ExternalInput").ap(),
        "b2": nc.dram_tensor("b2", (WIDTH,), f32, kind="ExternalInput").ap(),
        "out": nc.dram_tensor("out", (bpc, 4, WIDTH), f32,
                              kind="ExternalOutput").ap(),
    }

    with tile.TileContext(nc) as tc:
        _emit(nc, tc, io, bpc, b_chunk)

    nc.compile()
    return nc


def _prep_weights(W1, W2):
    bf16 = ml_dtypes.bfloat16
    W1 = np.asarray(W1, dtype=np.float32)
    W2 = np.asarray(W2, dtype=np.float32)
    # w1_h[mid_i, p, kt, m] = W1[kt*128+p, mid_i*128+m]
    w1_h = np.ascontiguousarray(
        W1.reshape(32, 128, 32, 128).transpose(2, 1, 0, 3).astype(bf16))
    w2_h = np.ascontiguousarray(W2.reshape(32, 128, WIDTH).astype(bf16))
    return w1_h, w2_h


def kernel(x, ln_scale, ln_offset, W1, b1, W2, b2):
    global LAST_RESULTS
    from concourse.bass_utils import run_bass_kernel_spmd

    x = np.ascontiguousarray(np.asarray(x, dtype=np.float32))
    ln_scale = np.asarray(ln_scale, dtype=np.float32)
    ln_offset = np.asarray(ln_offset, dtype=np.float32)
    b1 = np.asarray(b1, dtype=np.float32)
    b2 = np.asarray(b2, dtype=np.float32)
    w1_h, w2_h = _prep_weights(W1, W2)

    if "nc" not in _CACHE:
        _CACHE["nc"] = _build()
    nc = _CACHE["nc"]

    in_maps = []
    for i in range(N_CORES):
        in_maps.append({
            "x": np.ascontiguousarray(x[i * BPC:(i + 1) * BPC]),
            "ln_scale": ln_scale, "ln_offset": ln_offset,
            "w1": w1_h, "b1": b1, "w2": w2_h, "b2": b2,
        })

    res = run_bass_kernel_spmd(nc, in_maps, core_ids=list(range(N_CORES)),
                               trace=TRACE)
    LAST_RESULTS = res
    return np.concatenate([r["out"] for r in res.results], axis=0)


def bench(x, ln_scale, ln_offset, W1, b1, W2, b2, iters=10):
    """Time device-side execution with device-resident inputs (test-only).

    Returns (times_s, output) where output is the full gathered result from
    the last run (for cross-checking)."""
    import time as _time

    import jax
    from jax.experimental.shard_map import shard_map
    from jax.sharding import Mesh, NamedSharding, PartitionSpec

    import concourse.mybir as mybir
    from concourse import bass2jax

    x = np.ascontiguousarray(np.asarray(x, dtype=np.float32))
    w1_h, w2_h = _prep_weights(W1, W2)
    vals = {
        "x": x,
        "ln_scale": np.asarray(ln_scale, np.float32),
        "ln_offset": np.asarray(ln_offset, np.float32),
        "w1": w1_h, "b1": np.asarray(b1, np.float32),
        "w2": w2_h, "b2": np.asarray(b2, np.float32),
    }

    if "nc" not in _CACHE:
        _CACHE["nc"] = _build()
    nc = _CACHE["nc"]
    bass2jax.install_neuronx_cc_hook()

    partition_name = (nc.partition_id_tensor.name
                      if nc.partition_id_tensor else None)
    in_names, out_names, out_avals, zero_outs = [], [], [], []
    for alloc in nc.m.functions[0].allocations:
        if not isinstance(alloc, mybir.MemoryLocationSet):
            continue
        name = alloc.memorylocations[0].name
        if alloc.kind == "ExternalInput":
            if name != partition_name:
                in_names.append(name)
        elif alloc.kind == "ExternalOutput":
            out_names.append(name)
            shape = tuple(alloc.tensor_shape)
            dt = mybir.dt.np(alloc.dtype)
            out_avals.append(jax.core.ShapedArray(shape, dt))
            zero_outs.append(np.zeros((N_CORES * shape[0],) + shape[1:], dt))
    n_params = len(in_names)
    all_in_names = tuple(in_names) + tuple(out_names)
    if partition_name is not None:
        all_in_names = all_in_names + (partition_name,)

    def _body(*args):
        operands = list(args)
        if partition_name is not None:
            operands.append(bass2jax.partition_id_tensor())
        outs = bass2jax._bass_exec_p.bind(
            *operands,
            out_avals=tuple(out_avals),
            in_names=all_in_names,
            out_names=tuple(out_names),
            lowering_input_output_aliases=(),
            sim_require_finite=True,
            sim_require_nnan=True,
            nc=nc)
        return tuple(outs)

    devices = jax.devices()[:N_CORES]
    mesh = Mesh(np.asarray(devices), ("core",))
    spec = PartitionSpec("core")
    n_args = n_params + len(out_names)
    sharded = jax.jit(
        shard_map(_body, mesh=mesh, in_specs=(spec,) * n_args,
                  out_specs=(spec,) * len(out_names), check_rep=False),
        keep_unused=True)

    # per-core shards concatenated on axis 0 (weights tiled across cores)
    concat_in = []
    for name in in_names:
        v = vals[name]
        if name == "x":
            concat_in.append(v)
        else:
            concat_in.append(np.tile(v, (N_CORES,) + (1,) * (v.ndim - 1)))
    sharding = NamedSharding(mesh, spec)
    dev_in = [jax.device_put(a, sharding) for a in concat_in + zero_outs]

    r = sharded(*dev_in)
    jax.block_until_ready(r)
    times = []
    for _ in range(iters):
        t0 = _time.perf_counter()
        r = sharded(*dev_in)
        jax.block_until_ready(r)
        times.append(_time.perf_counter() - t0)
    out = np.asarray(r[0])
    return times, out


# revision 57
# speedup vs baseline: 28.4252x; 28.4252x over previous
# Trainium2 Bass kernel for nn_EquivariantBlock (LN -> rolled-concat -> MLP -> residual).
#
# Math (see reference):
#   y = LayerNorm(x) over width
#   h[b,k,:]   = sum_j y[b,(j+k)%4,:] @ W1[j*W:(j+1)*W, :] + b1      (j = (c-k)%4 for channel c)
#   out[b,k,:] = gelu_tanh(h) @ W2 + b2
#   return x + out
#
# Strategy: pure data-parallel over batch across 8 NeuronCores (weights replicated).
# Per core (1024 batch rows), process chunks of 256 rows:
#   A) LN in token-major layout (bn_stats/bn_aggr), cast bf16, DMA-transpose to
#      feature-major yT[feat128, c, ft, tok256].
#   B) For each mid tile (32): mm1 accumulates 32 K-subtiles into PSUM, where the
#      "roll" is just W1-block reindexing (j = (c-k)%4); gelu+b1 on ScalarE from
#      PSUM into a full-mid hT[mid128, k, mid_i, tok].
#   C) mm2: for each (k, out-tile): accumulate all 32 mid tiles in PSUM, then
#      fused residual+bias: out = x + psum + b2, DMA out.
# Matmuls run in bf16 (fp32 accumulation in PSUM).

import numpy as np
import ml_dtypes

BATCH = 8192
WIDTH = 1024
MID = 4096
N_CORES = 8
BPC = BATCH // N_CORES  # batch rows per core
LN_EPS = 1e-5

_CACHE = {}

# test.py hooks
TRACE = False
LAST_RESULTS = None


def _emit(nc, tc, io, bpc, b_chunk, gelu_func=None):
    """Emit the per-core program. io: dict of DRAM APs (x, ln_scale, ln_offset,
    w1, b1, w2, b2, out)."""
    from contextlib import ExitStack

    import concourse.bass as bass
    import concourse.mybir as mybir

    f32 = mybir.dt.float32
    bf16 = mybir.dt.bfloat16

    x, out = io["x"], io["out"]
    w1, b1, w2, b2 = io["w1"], io["b1"], io["w2"], io["b2"]

    if gelu_func is None:
        gelu_func = mybir.ActivationFunctionType.Gelu_apprx_tanh

    MID_BLK = 8   # mid tiles per block (32 total)
    n_blk = 32 // MID_BLK
    n_chunks = bpc // b_chunk
    n_bt = b_chunk // 128  # 128-row tiles per chunk per channel
    sub_op = mybir.AluOpType.subtract
    add_op = mybir.AluOpType.add
    mult_op = mybir.AluOpType.mult

    def bcast_ap(src_ap, parts=128):
        # DRAM AP replicated across `parts` partitions (stride-0 partition dim).
        return bass.AP(tensor=src_ap.tensor, offset=src_ap.offset,
                       ap=[[0, parts]] + [list(d) for d in src_ap.ap])

    with ExitStack() as ctx:
        singles = ctx.enter_context(tc.tile_pool(name="singles", bufs=1))
        xpool = ctx.enter_context(tc.tile_pool(name="xpool", bufs=4))
        ynorm = ctx.enter_context(tc.tile_pool(name="ynorm", bufs=5))
        ypool = ctx.enter_context(tc.tile_pool(name="ypool", bufs=5))
        stats = ctx.enter_context(tc.tile_pool(name="stats", bufs=4))
        yT_pool = ctx.enter_context(tc.tile_pool(name="yT", bufs=1))
        hT_pool = ctx.enter_context(tc.tile_pool(name="hT", bufs=1))
        hall_pool = ctx.enter_context(tc.tile_pool(name="hall", bufs=2))
        idft = ctx.enter_context(tc.tile_pool(name="idft", bufs=2))
        w1_pool = ctx.enter_context(tc.tile_pool(name="w1p", bufs=3))
        w2_pool = ctx.enter_context(tc.tile_pool(name="w2p", bufs=1))
        cp_pool = ctx.enter_context(tc.tile_pool(name="cp", bufs=3))
        dram = ctx.enter_context(tc.tile_pool(name="dram", bufs=2, space="DRAM"))
        psum1 = ctx.enter_context(tc.tile_pool(name="psum1", bufs=4, space="PSUM"))
        psum2 = ctx.enter_context(tc.tile_pool(name="psum2", bufs=4, space="PSUM"))

        # --- constants ---
        # (ln_scale/ln_offset are folded into w1/b1 on the host and are not
        # device inputs at all.)
        b2_bc = singles.tile([128, WIDTH], f32)
        nc.gpsimd.dma_start(out=b2_bc[:], in_=bcast_ap(b2))
        b1_p = singles.tile([128, 32], f32)
        nc.sync.dma_start(out=b1_p[:], in_=b1.rearrange("(o p) -> p o", p=128))
        eps_t = singles.tile([128, 1], f32)
        nc.vector.memset(eps_t[:], LN_EPS)

        def emit_finalize(b0, out_acc, ks=range(4)):
            # out = x + out_acc + b2
            for k in ks:
                for sub in range(n_bt):
                    r0 = b0 + sub * 128
                    xr = xpool.tile([128, WIDTH], f32, name="xr", tag="xt")
                    nc.scalar.dma_start(out=xr[:], in_=x[r0:r0 + 128, k, :])
                    oa = xpool.tile([128, WIDTH], f32, name="oa", tag="xt")
                    nc.scalar.dma_start(
                        out=oa[:], in_=out_acc[k, sub * 128:(sub + 1) * 128, :])
                    nc.vector.tensor_add(out=xr[:], in0=xr[:], in1=oa[:])
                    nc.vector.tensor_add(out=xr[:], in0=xr[:], in1=b2_bc[:])
                    nc.scalar.dma_start(out=out[r0:r0 + 128, k, :], in_=xr[:])

        pending_final = None
        for chunk in range(n_chunks):
            b0 = chunk * b_chunk

            # prefetch first w1 slices so PE can start the mid loop immediately
            pending_w1 = {}
            for mid_i in range(2):
                w1t = w1_pool.tile([128, 5, 8, 128], bf16, name="w1t", tag="w1t")
                nc.sync.dma_start(out=w1t[:], in_=w1[mid_i])
                pending_w1[mid_i] = w1t

            # ---------- Phase A: LayerNorm + Z4-DFT + transpose ----------
            # The roll-matmul is a cyclic correlation over the 4 channels:
            #   h[k] = sum_c y_c @ W1_{(c-k)%4}
            # 4-point DFT: device computes the 4 real components
            #   u0=y0+y1+y2+y3, u2=y0-y1+y2-y3, a=y0-y2, hh=y1-y3
            # and matmuls them against host-precombined weights (6 real block
            # matmuls instead of 16). Stage-major across the 4 channels of
            # each row-tile so DVE<->ACT latency doesn't serialize.
            yT = yT_pool.tile([128, 4, 8, b_chunk], bf16)
            for bt in range(n_bt):
                r0 = b0 + bt * 128
                xts, yns = [], []
                st = stats.tile([128, 4, 2, 6], f32)
                mv4 = stats.tile([128, 4, 2], f32)
                for c in range(4):
                    xt = xpool.tile([128, WIDTH], f32, name="xt", tag="xt")
                    nc.sync.dma_start(out=xt[:], in_=x[r0:r0 + 128, c, :])
                    nc.vector.bn_stats(out=st[:, c, 0, :], in_=xt[:, 0:512])
                    nc.vector.bn_stats(out=st[:, c, 1, :], in_=xt[:, 512:1024])
                    nc.vector.bn_aggr(out=mv4[:, c, :], in_=st[:, c, :, :])
                    xts.append(xt)
                rstd4 = stats.tile([128, 4], f32)
                nc.scalar.activation(
                    out=rstd4[:], in_=mv4[:, :, 1],
                    func=mybir.ActivationFunctionType.Sqrt,
                    bias=eps_t[:], scale=1.0)
                nc.vector.reciprocal(out=rstd4[:], in_=rstd4[:])
                nmr4 = stats.tile([128, 4], f32)
                nc.vector.tensor_tensor(nmr4[:], mv4[:, :, 0], rstd4[:],
                                        mult_op)
                nc.vector.tensor_scalar_mul(nmr4[:], nmr4[:], -1.0)
                for c in range(4):
                    yn = ynorm.tile([128, WIDTH], f32)
                    nc.scalar.activation(
                        out=yn[:], in_=xts[c][:],
                        func=mybir.ActivationFunctionType.Identity,
                        bias=nmr4[:, c:c + 1], scale=rstd4[:, c:c + 1])
                    yns.append(yn)
                # Z4-DFT components on gpsimd (SBUF-only) to keep DVE short;
                # e/f overwrite yn0/yn1 in place
                a_bf = ypool.tile([128, WIDTH], bf16, name="a_bf", tag="comp")
                nc.vector.tensor_tensor(a_bf[:], yns[0][:], yns[2][:], sub_op)
                hh_bf = ypool.tile([128, WIDTH], bf16, name="hh_bf", tag="comp")
                nc.vector.tensor_tensor(hh_bf[:], yns[1][:], yns[3][:], sub_op)
                nc.vector.tensor_tensor(yns[0][:], yns[0][:], yns[2][:], add_op)
                nc.vector.tensor_tensor(yns[1][:], yns[1][:], yns[3][:], add_op)
                u0_bf = ypool.tile([128, WIDTH], bf16, name="u0_bf", tag="comp")
                nc.vector.tensor_tensor(u0_bf[:], yns[0][:], yns[1][:], add_op)
                u2_bf = ypool.tile([128, WIDTH], bf16, name="u2_bf", tag="comp")
                nc.vector.tensor_tensor(u2_bf[:], yns[0][:], yns[1][:], sub_op)
                for ci_, comp in enumerate([u0_bf, u2_bf, a_bf, hh_bf]):
                    for ft in range(8):
                        nc.sync.dma_start(
                            out=yT[:, ci_, ft, bt * 128:(bt + 1) * 128],
                            in_=comp[:, ft * 128:(ft + 1) * 128],
                            transpose=True)

            # finalize previous chunk after this chunk's LN is queued on DVE,
            # so DVE's in-order stream doesn't stall the next chunk's matmuls
            if pending_final is not None:
                emit_finalize(*pending_final)
                pending_final = None

            # ---------- Phase B/C interleaved over mid blocks ----------
            # out_acc accumulates mm2 partials across mid blocks in DRAM
            # (PSUM can't be DMA'd; VectorE copies psum->SBUF staging first).
            out_acc = dram.tile([4, b_chunk, WIDTH], f32)
            for blk in range(n_blk):
                w2t = w2_pool.tile([128, MID_BLK, WIDTH], bf16)
                nc.sync.dma_start(
                    out=w2t[:],
                    in_=w2[blk * MID_BLK:(blk + 1) * MID_BLK].rearrange(
                        "m p w -> p m w"))
                hT = hT_pool.tile([128, 4, MID_BLK, b_chunk], bf16)
                for ml in range(MID_BLK):
                    mid_i = blk * MID_BLK + ml
                    if mid_i in pending_w1:
                        w1t = pending_w1.pop(mid_i)
                    else:
                        w1t = w1_pool.tile([128, 5, 8, 128], bf16,
                                           name="w1t", tag="w1t")
                        nc.sync.dma_start(out=w1t[:], in_=w1[mid_i])
                    # DFT-domain matmuls: P0=u0@W0, P2=u2@W2,
                    # Qr=a@WR+hh@WI, Qi=a@WI+hh@(-WR)
                    plan = [[(0, 0)], [(1, 1)], [(2, 2), (3, 3)],
                            [(3, 2), (4, 3)]]
                    P = []
                    for pi, pairs in enumerate(plan):
                        p1 = psum1.tile([128, b_chunk], f32, name="p1",
                                        tag="p1")
                        n_tot = 8 * len(pairs)
                        n = 0
                        for wc, yc in pairs:
                            for kt in range(8):
                                nc.tensor.matmul(
                                    p1[:], w1t[:, wc, kt, :], yT[:, yc, kt, :],
                                    start=(n == 0), stop=(n == n_tot - 1))
                                n += 1
                        P.append(p1)
                    # inverse DFT on DVE (one staged copy; <=1 PSUM operand/op)
                    cp2 = idft.tile([128, b_chunk], f32, name="cp2", tag="cp2")
                    nc.vector.tensor_copy(out=cp2[:], in_=P[1][:])
                    s_t = idft.tile([128, b_chunk], f32, name="s_t", tag="s_t")
                    nc.vector.tensor_tensor(s_t[:], P[0][:], cp2[:], add_op)
                    d_t = idft.tile([128, b_chunk], f32, name="d_t", tag="d_t")
                    nc.vector.tensor_tensor(d_t[:], P[0][:], cp2[:], sub_op)
                    h_all = hall_pool.tile([128, 4, b_chunk], f32)
                    nc.vector.tensor_tensor(h_all[:, 0, :], P[2][:], s_t[:],
                                            add_op)
                    nc.vector.tensor_tensor(h_all[:, 2, :], s_t[:], P[2][:],
                                            sub_op)
                    nc.vector.tensor_tensor(h_all[:, 1, :], d_t[:], P[3][:],
                                            sub_op)
                    nc.vector.tensor_tensor(h_all[:, 3, :], d_t[:], P[3][:],
                                            add_op)
                    nc.scalar.activation(
                        out=hT[:, :, ml, :], in_=h_all[:],
                        func=gelu_func,
                        bias=b1_p[:, mid_i:mid_i + 1], scale=1.0)
                last_all = (chunk == n_chunks - 1) and (blk == n_blk - 1)
                for k in range(4):
                    for sub in range(n_bt):
                        p2s = []
                        cp = None if last_all else cp_pool.tile([128, WIDTH], f32)
                        for wh in range(2):
                            p2 = psum2.tile([128, 512], f32, name="p2", tag="p2")
                            for ml in range(MID_BLK):
                                nc.tensor.matmul(
                                    p2[:],
                                    hT[:, k, ml, sub * 128:(sub + 1) * 128],
                                    w2t[:, ml, wh * 512:(wh + 1) * 512],
                                    start=(ml == 0), stop=(ml == MID_BLK - 1))
                            if cp is not None:
                                nc.vector.tensor_copy(
                                    out=cp[:, wh * 512:(wh + 1) * 512], in_=p2[:])
                            else:
                                p2s.append(p2)
                        if cp is not None:
                            nc.gpsimd.dma_start(
                                out=out_acc[k, sub * 128:(sub + 1) * 128, :],
                                in_=cp[:],
                                accum_op=(mybir.AluOpType.bypass if blk == 0
                                          else mybir.AluOpType.add))
                        else:
                            # tail shortcut: finalize straight from PSUM
                            # (out_acc holds the first n_blk-1 blocks)
                            r0 = b0 + sub * 128
                            xr = xpool.tile([128, WIDTH], f32, name="xr",
                                            tag="xt")
                            nc.scalar.dma_start(out=xr[:],
                                                in_=x[r0:r0 + 128, k, :])
                            oa = xpool.tile([128, WIDTH], f32, name="oa",
                                            tag="xt")
                            nc.scalar.dma_start(
                                out=oa[:],
                                in_=out_acc[k, sub * 128:(sub + 1) * 128, :])
                            nc.vector.tensor_add(out=xr[:], in0=xr[:],
                                                 in1=oa[:])
                            nc.vector.tensor_add(out=xr[:, 0:512],
                                                 in0=xr[:, 0:512], in1=p2s[0][:])
                            nc.vector.tensor_add(out=xr[:, 512:1024],
                                                 in0=xr[:, 512:1024],
                                                 in1=p2s[1][:])
                            nc.vector.tensor_add(out=xr[:], in0=xr[:],
                                                 in1=b2_bc[:])
                            nc.sync.dma_start(out=out[r0:r0 + 128, k, :],
                                              in_=xr[:])
            if not (chunk == n_chunks - 1):
                pending_final = (b0, out_acc)
            else:
                pending_final = None


def _build(bpc=BPC, b_chunk=512):
    """Build + compile the Bass program (one NeuronCore's SPMD program)."""
    import concourse.mybir as mybir
    import concourse.tile as tile
    from concourse import bacc

    f32 = mybir.dt.float32
    bf16 = mybir.dt.bfloat16

    nc = bacc.Bacc("TRN2", target_bir_lowering=False, debug=False,
                   enable_asserts=False)

    io = {
        "x": nc.dram_tensor("x", (bpc, 4, WIDTH), f32, kind="ExternalInput").ap(),
        # host-precombined Z4-DFT weights:
        # w1[mid_i, p, comp, kt, m], comp in (W0, W2, WR, WI, -WR)
        "w1": nc.dram_tensor("w1", (32, 128, 5, 8, 128), bf16,
                             kind="ExternalInput").ap(),
        "b1": nc.dram_tensor("b1", (MID,), f32, kind="ExternalInput").ap(),
        # w2[ml, p, w] = W2[ml*128+p, w]
        "w2": nc.dram_tensor("w2", (32, 128, WIDTH), bf16,
                             kind="ExternalInput").ap(),
        "b2": nc.dram_tensor("b2", (WIDTH,), f32, kind="ExternalInput").ap(),
        "out": nc.dram_tensor("out", (bpc, 4, WIDTH), f32,
                              kind="ExternalOutput").ap(),
    }

    with tile.TileContext(nc) as tc:
        _emit(nc, tc, io, bpc, b_chunk)

    nc.compile()
    return nc


def _prep_weights(W1, W2, ln_scale, ln_offset, b1):
    """Fold LN scale/offset into W1/b1, factor the Z4 cyclic correlation
    through a 4-point DFT, and pretranspose for clean DMA.

    rolls = concat_j(y_j*s + o)  =>  h = sum_j yhat_j @ (diag(s) W1_j) + b1'
    with b1' = b1 + tile(o,4) @ W1.

    With A_j = diag(s) W1_j and device components u0,u2,a,hh:
      P0 = u0@(A0+A1+A2+A3)/4     P2 = u2@(A0-A1+A2-A3)/4
      Qr = a@(A0-A2)/2 + hh@(A1-A3)/2
      Qi = a@(A1-A3)/2 + hh@(-(A0-A2)/2)
      h0,h2 = P0+P2 +- Qr ;  h1,h3 = P0-P2 -+ Qi
    """
    bf16 = ml_dtypes.bfloat16
    W1 = np.asarray(W1, dtype=np.float32)
    W2 = np.asarray(W2, dtype=np.float32)
    s4 = np.tile(np.asarray(ln_scale, np.float32), 4)
    o4 = np.tile(np.asarray(ln_offset, np.float32), 4)
    b1_eff = (np.asarray(b1, np.float32) + o4 @ W1).astype(np.float32)
    A = (W1 * s4[:, None]).reshape(4, WIDTH, MID)
    W0 = (A[0] + A[1] + A[2] + A[3]) * 0.25
    W2f = (A[0] - A[1] + A[2] - A[3]) * 0.25
    WR = (A[0] - A[2]) * 0.5
    WI = (A[1] - A[3]) * 0.5
    comps = np.stack([W0, W2f, WR, WI, -WR])  # [5, 1024, 4096]
    # w1_h[mid_i, p, comp, kt, m] = comps[comp, kt*128+p, mid_i*128+m]
    w1_h = np.ascontiguousarray(
        comps.reshape(5, 8, 128, 32, 128)
        .transpose(3, 2, 0, 1, 4).astype(bf16))
    w2_h = np.ascontiguousarray(W2.reshape(32, 128, WIDTH).astype(bf16))
    return w1_h, w2_h, b1_eff


def kernel(x, ln_scale, ln_offset, W1, b1, W2, b2):
    global LAST_RESULTS
    from concourse.bass_utils import run_bass_kernel_spmd

    x = np.ascontiguousarray(np.asarray(x, dtype=np.float32))
    ln_scale = np.asarray(ln_scale, dtype=np.float32)
    ln_offset = np.asarray(ln_offset, dtype=np.float32)
    b2 = np.asarray(b2, dtype=np.float32)
    w1_h, w2_h, b1 = _prep_weights(W1, W2, ln_scale, ln_offset, b1)

    if "nc" not in _CACHE:
        _CACHE["nc"] = _build()
    nc = _CACHE["nc"]

    in_maps = []
    for i in range(N_CORES):
        in_maps.append({
            "x": np.ascontiguousarray(x[i * BPC:(i + 1) * BPC]),
            "w1": w1_h, "b1": b1, "w2": w2_h, "b2": b2,
        })

    res = run_bass_kernel_spmd(nc, in_maps, core_ids=list(range(N_CORES)),
                               trace=TRACE)
    LAST_RESULTS = res
    return np.concatenate([r["out"] for r in res.results], axis=0)


def bench(x, ln_scale, ln_offset, W1, b1, W2, b2, iters=10):
    """Time device-side execution with device-resident inputs (test-only).

    Returns (times_s, output) where output is the full gathered result from
    the last run (for cross-checking)."""
    import time as _time

    import jax
    from jax.experimental.shard_map import shard_map
    from jax.sharding import Mesh, NamedSharding, PartitionSpec

    import concourse.mybir as mybir
    from concourse import bass2jax

    x = np.ascontiguousarray(np.asarray(x, dtype=np.float32))
    w1_h, w2_h, b1_eff = _prep_weights(W1, W2, ln_scale, ln_offset, b1)
    vals = {
        "x": x,
        "w1": w1_h, "b1": b1_eff,
        "w2": w2_h, "b2": np.asarray(b2, np.float32),
    }

    if "nc" not in _CACHE:
        _CACHE["nc"] = _build()
    nc = _CACHE["nc"]
    bass2jax.install_neuronx_cc_hook()

    partition_name = (nc.partition_id_tensor.name
                      if nc.partition_id_tensor else None)
    in_names, out_names, out_avals, zero_outs = [], [], [], []
    for alloc in nc.m.functions[0].allocations:
        if not isinstance(alloc, mybir.MemoryLocationSet):
            continue
        name = alloc.memorylocations[0].name
        if alloc.kind == "ExternalInput":
            if name != partition_name:
                in_names.append(name)
        elif alloc.kind == "ExternalOutput":
            out_names.append(name)
            shape = tuple(alloc.tensor_shape)
            dt = mybir.dt.np(alloc.dtype)
            out_avals.append(jax.core.ShapedArray(shape, dt))
            zero_outs.append(np.zeros((N_CORES * shape[0],) + shape[1:], dt))
    n_params = len(in_names)
    all_in_names = tuple(in_names) + tuple(out_names)
    if partition_name is not None:
        all_in_names = all_in_names + (partition_name,)

    def _body(*args):
        operands = list(args)
        if partition_name is not None:
            operands.append(bass2jax.partition_id_tensor())
        outs = bass2jax._bass_exec_p.bind(
            *operands,
            out_avals=tuple(out_avals),
            in_names=all_in_names,
            out_names=tuple(out_names),
            lowering_input_output_aliases=(),
            sim_require_finite=True,
            sim_require_nnan=True,
            nc=nc)
        return tuple(outs)

    devices = jax.devices()[:N_CORES]
    mesh = Mesh(np.asarray(devices), ("core",))
    spec = PartitionSpec("core")
    n_args = n_params + len(out_names)
    sharded = jax.jit(
        shard_map(_body, mesh=mesh, in_specs=(spec,) * n_args,
                  out_specs=(spec,) * len(out_names), check_rep=False),
        keep_unused=True)

    # per-core shards concatenated on axis 0 (weights tiled across cores)
    concat_in = []
    for name in in_names:
        v = vals[name]
        if name == "x":
            concat_in.append(v)
        else:
            concat_in.append(np.tile(v, (N_CORES,) + (1,) * (v.ndim - 1)))
    sharding = NamedSharding(mesh, spec)
    dev_in = [jax.device_put(a, sharding) for a in concat_in + zero_outs]

    r = sharded(*dev_in)
    jax.block_until_ready(r)
    out = np.asarray(r[0])

    times = []
    for _ in range(iters):
        t0 = _time.perf_counter()
        r = sharded(*dev_in)
        jax.block_until_ready(r)
        times.append(_time.perf_counter() - t0)
    return times, out
